# revision 8
# baseline (speedup 1.0000x reference)
"""LSTM decoder w/ Luong attention + input feeding, Trainium2 Bass kernel.

T=64 steps, B=64, D=512, S=512, 2-layer LSTM, dot attention, input feed.
Sharding: data-parallel over batch, 8 cores x 8 batches, zero collectives.

v2: hardware For_i loop over the 64 time steps (64x smaller program ->
fast build/compile/NEFF-load), fully SBUF-resident working set (embT,
memory bank in both orientations, outputs accumulated in SBUF), fp16
operands everywhere (better accuracy than bf16 at the same byte cost;
the d-major score copy of the memory bank is derived on-device from the
s-major copy via PE transposes instead of being uploaded in f32).
"""

import os
import sys

sys.path.insert(0, "/opt/trn_rl_repo")

import numpy as np
import ml_dtypes

T_FULL, B_FULL, D, S, V = 64, 64, 512, 512, 32000
NC = 8
BL = B_FULL // NC  # 8 batches per core
G = 4 * D  # 2048
NK_D = D // 128  # 4
T_STEPS = int(os.environ.get("KERNEL_T", T_FULL))

BF16 = ml_dtypes.bfloat16
F16 = np.float16


def _build(T):
    import concourse.bass as bass
    import concourse.bacc as bacc
    import concourse.tile as tile
    from concourse import mybir
    from concourse.bass import ds
    from concourse.masks import make_identity

    nc = bacc.Bacc(None, target_bir_lowering=False)
    f32 = mybir.dt.float32
    f16 = mybir.dt.float16
    bf16 = mybir.dt.bfloat16
    AF = mybir.ActivationFunctionType

    PS = 128 // NC  # weight-shard partition rows per core
    embT_d = nc.dram_tensor("embT", [128, NK_D, T_FULL, BL], f16, kind="ExternalInput")
    wih0_d = nc.dram_tensor("wih0s", [PS, 2 * NK_D, G], f16, kind="ExternalInput")
    whh0_d = nc.dram_tensor("whh0s", [PS, NK_D, G], f16, kind="ExternalInput")
    wih1_d = nc.dram_tensor("wih1s", [PS, NK_D, G], f16, kind="ExternalInput")
    whh1_d = nc.dram_tensor("whh1s", [PS, NK_D, G], f16, kind="ExternalInput")
    wout_d = nc.dram_tensor("wouts", [PS, 2 * NK_D, D], f16, kind="ExternalInput")
    bias_d = nc.dram_tensor("bias01s", [1, 2 * G // NC], f16, kind="ExternalInput")
    memc_d = nc.dram_tensor("memc", [128, NK_D, BL, D], f16, kind="ExternalInput")
    mask_d = nc.dram_tensor("mask", [128, 2, S], bf16, kind="ExternalInput")
    decT_d = nc.dram_tensor("decT", [128, NK_D, T_FULL, BL], f16, kind="ExternalOutput")
    attT_d = nc.dram_tensor("attT", [128, NK_D, T_FULL, BL], f16, kind="ExternalOutput")

    with tile.TileContext(nc) as tc:
        with (
            tc.tile_pool(name="res", bufs=1) as res,
            tc.tile_pool(name="state", bufs=1) as state,
            tc.tile_pool(name="work", bufs=1) as work,
            tc.tile_pool(name="dram", bufs=1, space="DRAM") as dram,
            tc.tile_pool(name="pg", bufs=1, space="PSUM") as pg,
            tc.tile_pool(name="pg2", bufs=2, space="PSUM") as pg2,
            tc.tile_pool(name="pt", bufs=2, space="PSUM") as pt,
        ):
            # weights arrive sharded 1/8th per core; AllGather them on-device
            # (bounce through internal DRAM — collectives can't touch I/O
            # tensors directly), then load to SBUF
            def gather_load(in_d, full_shape, tag):
                bounce = dram.tile(list(in_d.shape), f16, tag=f"{tag}_bnc")
                nc.gpsimd.dma_start(bounce[:], in_d.ap())
                full = dram.tile(full_shape, f16, tag=f"{tag}_full")
                nc.gpsimd.collective_compute(
                    "AllGather", mybir.AluOpType.bypass,
                    replica_groups=[list(range(NC))],
                    ins=[bounce.opt()], outs=[full.opt()])
                sb = res.tile(full_shape, f16, tag=tag)
                nc.sync.dma_start(out=sb, in_=full[:])
                return sb

            wih0 = gather_load(wih0_d, [128, 2 * NK_D, G], "wih0")
            whh0 = gather_load(whh0_d, [128, NK_D, G], "whh0")
            wih1 = gather_load(wih1_d, [128, NK_D, G], "wih1")
            whh1 = gather_load(whh1_d, [128, NK_D, G], "whh1")
            wout = gather_load(wout_d, [128, 2 * NK_D, D], "wout")
            bias01 = gather_load(bias_d, [1, 2 * G], "bias01")
            memc = res.tile([128, NK_D, BL, D], f16)
            nc.sync.dma_start(out=memc, in_=memc_d.ap())
            mask = res.tile([128, 2, S], bf16)
            nc.sync.dma_start(out=mask, in_=mask_d.ap())
            embT = res.tile([128, NK_D, T_FULL, BL], f16)
            nc.sync.dma_start(out=embT, in_=embT_d.ap())
            eye32 = res.tile([128, 128], f32)
            make_identity(nc, eye32)
            eye16 = res.tile([128, 128], f16)
            make_identity(nc, eye16)
            ones = res.tile([1, BL], f16)
            nc.vector.memset(ones, 1.0)

            # derive d-major copy of the memory bank (for score matmuls)
            # from the s-major copy via PE transposes
            memT = res.tile([128, NK_D, BL, S], f16)
            for b in range(BL):
                for sk in range(NK_D):
                    tpm = pt.tile([128, NK_D, 128], f16, tag="tp")
                    for dk in range(NK_D):
                        nc.tensor.transpose(
                            tpm[:, dk, :], memc[:, sk, b, dk * 128 : (dk + 1) * 128],
                            eye16)
                    nc.vector.tensor_copy(
                        memT[:, :, b, sk * 128 : (sk + 1) * 128], tpm)

            c0 = state.tile([BL, D], f32)
            c1 = state.tile([BL, D], f32)
            h0T = state.tile([128, NK_D, BL], f16)
            h1T = state.tile([128, NK_D, BL], f16)
            h1Td2 = state.tile([128, NK_D, 2, BL], f16)
            feedT = state.tile([128, NK_D, BL], f16)
            decT = state.tile([128, NK_D, T_FULL, BL], f16)
            attT = state.tile([128, NK_D, T_FULL, BL], f16)
            for t_ in (c0, c1, h0T, h1T, h1Td2, feedT, decT, attT):
                nc.vector.memset(t_, 0.0)
            # pre-zero the psc/cxs slots so garbage partitions can never be NaN
            psc0 = work.tile([128, 2, S], f32, tag="p")
            nc.vector.memset(psc0, 0.0)
            cxs0 = work.tile([128, 2, D], f32, tag="cxs")
            nc.vector.memset(cxs0, 0.0)

            IFO = 3 * D

            def transpose_8xD(src_sb, outs, dup_out=None):
                """src [8,512] f32 SBUF -> each out tile [128,NK_D,8] (cast).
                dup_out: [128,NK_D,2,BL] tile receiving doubled columns.
                Returns the PSUM transpose tile."""
                tp = pt.tile([128, NK_D, BL], f32, tag="tp")
                for k in range(NK_D):
                    nc.tensor.transpose(
                        tp[:, k, :], src_sb[:, k * 128 : (k + 1) * 128],
                        eye32[0:BL, 0:BL],
                    )
                for o in outs:
                    nc.vector.tensor_copy(o, tp)
                if dup_out is not None:
                    tv = tp[:, :, :]
                    dup = bass.AP(tensor=tv.tensor, offset=tv.offset,
                                  ap=[tv.ap[0], tv.ap[1], [0, 2], tv.ap[2]])
                    nc.vector.tensor_copy(dup_out, dup)
                return tp

            def lstm_cell(gps, cprev, houts, dup_out=None):
                sig = work.tile([BL, IFO], f32, tag="sig")
                nc.scalar.activation(sig, gps[:, 0:IFO], AF.Sigmoid)
                tg = work.tile([BL, D], f32, tag="tg")
                nc.scalar.activation(tg, gps[:, IFO:G], AF.Tanh)
                fc = work.tile([BL, D], f32, tag="tc")
                nc.vector.tensor_mul(fc, sig[:, D : 2 * D], cprev)
                ig = work.tile([BL, D], f32, tag="h")
                nc.vector.tensor_mul(ig, sig[:, 0:D], tg)
                nc.vector.tensor_add(cprev, fc, ig)
                tc_ = work.tile([BL, D], f32, tag="tc")
                nc.scalar.activation(tc_, cprev, AF.Tanh)
                h = work.tile([BL, D], f32, tag="h")
                nc.vector.tensor_mul(h, sig[:, 2 * D : IFO], tc_)
                return transpose_8xD(h, houts, dup_out=dup_out)

            with tc.For_i(0, T, 1) as t:
                # stage this step's embedding column once (single dynamic AP)
                et = work.tile([128, NK_D, BL], f16, tag="et")
                nc.vector.tensor_copy(et, embT[:, :, ds(t, 1), :].squeeze(2))

                # ===== layer-0 gates: [emb;feed;1] @ [Wih0.T;b0] + h0@Whh0.T
                g0 = pg.tile([BL, G], f32, tag="gates")
                for n in range(4):
                    nsl = slice(n * 512, (n + 1) * 512)
                    for k in range(NK_D):
                        nc.tensor.matmul(g0[:, nsl], et[:, k, :],
                                         wih0[:, k, nsl], start=(k == 0), stop=False)
                    for k in range(NK_D):
                        nc.tensor.matmul(g0[:, nsl], feedT[:, k, :],
                                         wih0[:, NK_D + k, nsl], start=False, stop=False)
                    for k in range(NK_D):
                        nc.tensor.matmul(g0[:, nsl], h0T[:, k, :],
                                         whh0[:, k, nsl], start=False, stop=False)
                    nc.tensor.matmul(g0[:, nsl], ones, bias01[:, nsl],
                                     start=False, stop=True)
                lstm_cell(g0, c0, [h0T])

                # ===== layer-1 gates
                g1 = pg.tile([BL, G], f32, tag="gates")
                for n in range(4):
                    nsl = slice(n * 512, (n + 1) * 512)
                    for k in range(NK_D):
                        nc.tensor.matmul(g1[:, nsl], h0T[:, k, :],
                                         wih1[:, k, nsl], start=(k == 0), stop=False)
                    for k in range(NK_D):
                        nc.tensor.matmul(g1[:, nsl], h1T[:, k, :],
                                         whh1[:, k, nsl], start=False, stop=False)
                    nc.tensor.matmul(g1[:, nsl], ones,
                                     bias01[:, G + n * 512 : G + (n + 1) * 512],
                                     start=False, stop=True)
                lstm_cell(g1, c1, [h1T], dup_out=h1Td2)

                # ===== attention scores. Rotated dup lhsT puts batch b's row
                # at partition 0; spread out to partition 32j, half u=b//4.
                psc = work.tile([128, 2, S], f32, tag="p")
                for b in range(BL):
                    u, j = b // 4, b % 4
                    ob = pg2.tile([BL, S], f32, tag="sc8")
                    for k in range(NK_D):
                        nc.tensor.matmul(
                            ob, h1Td2[:, k, :, :].rearrange("p a b -> p (a b)")[
                                :, b : b + BL],
                            memT[:, k, b, :],
                            start=(k == 0), stop=(k == NK_D - 1))
                    if b % 2 == 0:
                        nc.vector.tensor_copy(psc[32 * j : 32 * j + 1, u, :],
                                              ob[0:1, :])
                    else:
                        nc.scalar.copy(psc[32 * j : 32 * j + 1, u, :], ob[0:1, :])
                nc.vector.tensor_add(psc, psc, mask)
                nmx = work.tile([128, 2], f32, tag="nmx")
                nc.vector.tensor_reduce(nmx, psc, axis=mybir.AxisListType.X,
                                        op=mybir.AluOpType.max, negate=True)
                ssum = work.tile([128, 2], f32, tag="ssum")
                for u in range(2):
                    nc.scalar.activation(psc[:, u, :], psc[:, u, :], AF.Exp,
                                         bias=nmx[:, u : u + 1], scale=1.0,
                                         accum_out=ssum[:, u : u + 1])
                rs = work.tile([128, 2], f32, tag="rs")
                nc.vector.reciprocal(rs, ssum)
                for u in range(2):
                    nc.vector.tensor_scalar_mul(psc[:, u, :], in0=psc[:, u, :],
                                                scalar1=rs[:, u : u + 1])
                # transpose spread p; gather+dup to pT2 [128,NK_D,2*BL] f16,
                # gather (no dup) to pa for the attention output buffer
                pT2 = work.tile([128, NK_D, 2, BL], f16, tag="pT2")
                pa = work.tile([128, NK_D, BL], f16, tag="pa")
                for k in range(NK_D):
                    tk = pt.tile([128, 2, 128], f32, tag="tp")
                    for u in range(2):
                        nc.tensor.transpose(
                            tk[:, u, :], psc[:, u, 128 * k : 128 * (k + 1)],
                            eye32)
                    tv = tk[:, :, :]
                    gat = bass.AP(tensor=tv.tensor, offset=tv.offset,
                                  ap=[tv.ap[0], [0, 2], [128, 2], [32, 4]])
                    nc.vector.tensor_copy(pT2[:, k], gat)
                    gat2 = bass.AP(tensor=tv.tensor, offset=tv.offset,
                                   ap=[tv.ap[0], [128, 2], [32, 4]])
                    nc.scalar.copy(pa[:, k], gat2)
                nc.vector.tensor_copy(attT[:, :, ds(t, 1), :].squeeze(2), pa)

                # ===== context from the SBUF-resident s-major memory bank
                cxs = work.tile([128, 2, D], f32, tag="cxs")
                for b in range(BL):
                    u, j = b // 4, b % 4
                    cb = pg2.tile([BL, D], f32, tag="sc8")
                    for k in range(NK_D):
                        nc.tensor.matmul(
                            cb, pT2[:, k, :, :].rearrange("p a b -> p (a b)")[
                                :, b : b + BL],
                            memc[:, k, b, :],
                            start=(k == 0), stop=(k == NK_D - 1))
                    if b % 2 == 0:
                        nc.vector.tensor_copy(cxs[32 * j : 32 * j + 1, u, :],
                                              cb[0:1, :])
                    else:
                        nc.scalar.copy(cxs[32 * j : 32 * j + 1, u, :], cb[0:1, :])
                cxT = work.tile([128, NK_D, 2, 128], f16, tag="xT")
                for k in range(NK_D):
                    tk = pt.tile([128, 2, 128], f32, tag="tp")
                    for u in range(2):
                        nc.tensor.transpose(
                            tk[:, u, :], cxs[:, u, 128 * k : 128 * (k + 1)],
                            eye32)
                    nc.vector.tensor_copy(cxT[:, k], tk)

                # ===== output projection + tanh
                # lhsT cols (u,j) at free offset 32j of half u -> M=8 in b order
                ah = pt.tile([BL, D], f32, tag="tp")
                for k in range(NK_D):
                    cv = cxT[:, k, :, :]
                    lv = bass.AP(tensor=cv.tensor, offset=cv.offset,
                                 ap=[cv.ap[0], [128, 2], [32, 4]])
                    nc.tensor.matmul(ah[:, :], lv, wout[:, k, :],
                                     start=(k == 0), stop=False)
                for k in range(NK_D):
                    nc.tensor.matmul(ah[:, :], h1T[:, k, :], wout[:, NK_D + k, :],
                                     start=False, stop=(k == NK_D - 1))
                af = work.tile([BL, D], f32, tag="h")
                nc.scalar.activation(af, ah, AF.Tanh)
                tpf = transpose_8xD(af, [feedT])
                nc.vector.tensor_copy(decT[:, :, ds(t, 1), :].squeeze(2), tpf)

            nc.sync.dma_start(out=decT_d.ap(), in_=decT)
            nc.sync.dma_start(out=attT_d.ap(), in_=attT)
    nc.compile()
    return nc


def kernel(tokens, memory_bank, memory_lengths, emb_table,
           Wih0, Whh0, bih0, bhh0, Wih1, Whh1, bih1, bhh1, Wout):
    import concourse.tile_utils as tile_utils
    from concourse.bass_utils import run_bass_kernel_spmd

    tile_utils.max_sbuf_usage = 206 * 1024

    tokens = np.asarray(tokens)
    memory_bank = np.asarray(memory_bank, dtype=np.float32)
    memory_lengths = np.asarray(memory_lengths)
    f32 = np.float32

    # gate reorder [i,f,g,o] -> [i,f,o,g]
    perm = np.concatenate([np.arange(0, 2 * D), np.arange(3 * D, 4 * D),
                           np.arange(2 * D, 3 * D)])
    Wih0p, Whh0p = np.asarray(Wih0, f32)[perm], np.asarray(Whh0, f32)[perm]
    Wih1p, Whh1p = np.asarray(Wih1, f32)[perm], np.asarray(Whh1, f32)[perm]
    b0 = (np.asarray(bih0, f32) + np.asarray(bhh0, f32))[perm]
    b1 = (np.asarray(bih1, f32) + np.asarray(bhh1, f32))[perm]
    bias01 = np.concatenate([b0, b1])[None, :].astype(F16)

    def wT(w, nk):
        # [O, K] -> [128, nk, O] f16 (K = nk*128, partition-major)
        return w.T.reshape(nk, 128, w.shape[0]).transpose(1, 0, 2).astype(F16)

    wih0T = wT(Wih0p, 2 * NK_D)
    whh0T, wih1T, whh1T = wT(Whh0p, NK_D), wT(Wih1p, NK_D), wT(Whh1p, NK_D)
    woutT = wT(np.asarray(Wout, f32), 2 * NK_D)
    emb = np.asarray(emb_table, f32)[tokens.astype(np.int64)]  # [T,B,D]

    nc = _build(T_STEPS)

    # per-core views; run_bass_kernel_spmd's concat does the single copy
    embT_all = emb.astype(F16).reshape(T_FULL, NC, BL, NK_D, 128) \
        .transpose(1, 4, 3, 0, 2)  # [NC,128,NK_D,T,BL]
    memc_all = memory_bank.astype(F16).reshape(NK_D, 128, NC, BL, D) \
        .transpose(2, 1, 0, 3, 4)  # [NC,128,NK_D,BL,D]
    lens = memory_lengths.astype(np.int64)
    mrow = np.where(np.arange(S)[None, :] < lens[:, None], 0.0,
                    -1e9).astype(BF16)  # [B,S]
    mask_all = np.full((NC, 128, 2, S), -1e9, dtype=BF16)
    bidx = np.arange(B_FULL)
    mask_all[bidx // BL, 32 * ((bidx % BL) % 4), (bidx % BL) // 4, :] = mrow

    PS = 128 // NC
    BSH = 2 * G // NC
    in_maps = []
    for c in range(NC):
        psl = slice(c * PS, (c + 1) * PS)
        in_maps.append(dict(
            embT=embT_all[c], memc=memc_all[c], mask=mask_all[c],
            wih0s=wih0T[psl], whh0s=whh0T[psl], wih1s=wih1T[psl],
            whh1s=whh1T[psl], wouts=woutT[psl],
            bias01s=bias01[:, c * BSH : (c + 1) * BSH]))

    res = run_bass_kernel_spmd(
        nc, in_maps, core_ids=list(range(NC)),
        trace=bool(int(os.environ.get("KERNEL_TRACE", "0"))))
    dec = np.concatenate(
        [r["decT"].transpose(2, 3, 1, 0).reshape(T_FULL, BL, D)
         for r in res.results], axis=1).astype(f32)
    att = np.concatenate(
        [r["attT"].transpose(2, 3, 1, 0).reshape(T_FULL, BL, S)
         for r in res.results], axis=1).astype(f32)
    globals()["_last_results"] = res
    return dec, att
